# revision 42
# baseline (speedup 1.0000x reference)
"""CMADE ensemble kernel for 8 TRN2 NeuronCores.

Problem: B=16 binary-masked 4-layer MLPs (96 -> 1024 -> 1024 -> 1024 -> 64)
over the same N=4096 batch; output = mean over the 16 masks.

Strategy: data-parallel over the batch N -- each core takes 512 rows and
runs all 16 masked MLPs on them, accumulating the final-layer outputs of
all 16 masks into a single PSUM tile; no inter-core collective is needed.

v4 design:
- fp8 slice allocation chosen from a measured error-per-slice frontier
  (the host simulator reproduces hardware rel-err to ~0.2%): every mask
  runs L1 with k-slices 0:4 as fp8 DoubleRow pairs and 4:8 fp16; the
  first UPG masks run L2 fully fp8 (4 DR matmuls), the rest (4,4).
  Mid-layer matmuls: 48 + 32/48 per mask vs 104 in the v2 baseline.
- Per-mask calibrated corrections, computed on gaussian samples (input-
  independent): a per-neuron gain folded into the pre-quantization
  weights, a per-neuron bias folded into the drain biases, and a final
  output correction folded into b3 (applied on host). Together they
  remove ~20-25% of the quantization error variance, which is what pays
  for the larger fp8 coverage at the same end-to-end error as v2.
- PE DVFS warmup matmuls fill the startup DMA window.
- Weight streams ride 3 independent DMA queues (sync: mw1 streams,
  scalar: startup only, gpsimd SWDGE: mw0/mw2/mw3); masks 0/1 chunked so
  first consumers start early.
- Finalize: the two column-group halves of the accumulator psum are
  fp16-copied and DMAed out; the host sums halves, scales and adds b3.
"""

import numpy as np
import ml_dtypes

from concourse import bacc
import concourse.bass as bass
import concourse.mybir as mybir
import concourse.tile as tile
from concourse.bass_utils import run_bass_kernel_spmd

F16 = np.float16
E4 = ml_dtypes.float8_e4m3

N = 4096
B = 16
NCORES = 8
NLOC = N // NCORES           # 512 batch rows per core
D_IN = 96
H = 1024
D_OUT = 64
KT = H // 128                # 8 k-tiles for the 1024-wide dims
NF1 = 4                      # L1 fp8 k-slices (all masks)
UPG = 6                      # leading masks with fully-fp8 L2
SA0 = 8.0                    # a0 fp8 storage scale (folded into mw0 cols)
SW1 = 4.0                    # L1 fp8 weight scale
SA1 = SA0 * SW1              # psum1 units
SW2 = 64.0                   # L2 weight scale (fp8 and fp16 parts)
NWARM = 12                   # PE DVFS warmup matmuls (fp32, free=40)
CALIB_M = 1024               # calibration samples (gaussian, fixed seed)

TRACE = False
LAST_RESULT = None

_CACHE = {}


def _ensure_ntff_hook():
    """The agent image's antenv lacks axon_hooks; reconstruct the NTFF
    profile hook from trn_agent_boot so trace=True yields exec_time_ns."""
    import sys as _sys
    import types
    try:
        from antenv import axon_hooks  # noqa: F401
        return
    except ImportError:
        pass
    import antenv
    import concourse.bass_utils as _bu
    _bu.upload_artifacts = lambda tmpdir: tmpdir  # zero-egress container
    holder = {}
    mod = types.ModuleType("antenv.axon_hooks")
    mod.set_axon_ntff_profile_hook = lambda h: holder.__setitem__("h", h)
    mod.get_axon_ntff_profile_hook = lambda: holder.get("h")
    _sys.modules["antenv.axon_hooks"] = mod
    antenv.axon_hooks = mod
    from trn_agent_boot.trn_boot import _ntff_profile_via_ctypes
    mod.set_axon_ntff_profile_hook(
        _ntff_profile_via_ctypes("/opt/axon/libaxon_pjrt.so"))


def _build_graph():
    f32 = mybir.dt.float32
    f16 = mybir.dt.float16
    f8 = mybir.dt.float8e4
    nc = bacc.Bacc("TRN2", target_bir_lowering=False, debug=False,
                   num_devices=NCORES)

    # ---- I/O ----
    xyT_d = nc.dram_tensor("xyT", [128, NLOC], f16, kind="ExternalInput")
    mw0_d = nc.dram_tensor("mw0", [B, 128, H], f16, kind="ExternalInput")
    mw1f_d = nc.dram_tensor("mw1f", [B, 128, NF1, H], f8,
                            kind="ExternalInput")
    mw1h_d = nc.dram_tensor("mw1h", [B, 128, KT - NF1, H], f16,
                            kind="ExternalInput")
    mw2fA_d = nc.dram_tensor("mw2fA", [UPG, 128, KT, H], f8,
                             kind="ExternalInput")
    mw2fB_d = nc.dram_tensor("mw2fB", [B - UPG, 128, 4, H], f8,
                             kind="ExternalInput")
    mw2hB_d = nc.dram_tensor("mw2hB", [B - UPG, 128, 4, H], f16,
                             kind="ExternalInput")
    mw3_d = nc.dram_tensor("mw3", [B, 128, KT, D_OUT], f16,
                           kind="ExternalInput")
    # bias layout: cols 0:8 = b0 (first NF1 cols *SA0); per mask b:
    # 8+16b+m = SA1*(b1+d1), 8+16b+8+m = SA1*SW2*(b2+d2)
    bias_d = nc.dram_tensor("biasp", [128, 8 + 16 * B], f32,
                            kind="ExternalInput")
    out_d = nc.dram_tensor("out", [128, NLOC], f16, kind="ExternalOutput")

    relu = mybir.ActivationFunctionType.Relu
    add_op = mybir.AluOpType.add
    max_op = mybir.AluOpType.max
    DR = mybir.MatmulPerfMode.DoubleRow

    from contextlib import ExitStack
    with tile.TileContext(nc) as tc, ExitStack() as ctx:
        const = ctx.enter_context(tc.tile_pool(name="const", bufs=1))
        mw0p = ctx.enter_context(tc.tile_pool(name="mw0", bufs=2))
        mw1fp = ctx.enter_context(tc.tile_pool(name="mw1f", bufs=2))
        mw1hp = ctx.enter_context(tc.tile_pool(name="mw1h", bufs=2))
        mw2p = ctx.enter_context(tc.tile_pool(name="mw2", bufs=2))
        mw3p = ctx.enter_context(tc.tile_pool(name="mw3", bufs=2))
        apool = ctx.enter_context(tc.tile_pool(name="act", bufs=2))
        pspool = ctx.enter_context(tc.tile_pool(name="ps", bufs=7,
                                                space="PSUM"))
        ps3pool = ctx.enter_context(tc.tile_pool(name="ps3", bufs=1,
                                                 space="PSUM"))
        finp = ctx.enter_context(tc.tile_pool(name="fin", bufs=2))

        # ---- startup-critical loads; the small bias block leads the sync
        # queue so the PE warmup has data at the earliest possible moment
        bt = const.tile([128, 8 + 16 * B], f32, tag="bt")
        nc.sync.dma_start(bt[:, 0:40], bias_d[:, 0:40])
        xyT = const.tile([128, NLOC], f16, tag="xyT")
        nc.sync.dma_start(xyT[:], xyT_d[:])
        nc.gpsimd.dma_start(bt[:, 40:], bias_d[:, 40:])

        # ---- per-mask weight streams; masks 0/1 chunked across the sync
        # and scalar queues so early k-slices land before their consumers
        def fetch(b):
            mw0t = mw0p.tile([128, H], f16, tag="mw0", bufs=4,
                             name=f"mw0_{b}")
            if b == 0:
                nc.scalar.dma_start(mw0t[:, 0:256], mw0_d[b][:, 0:256])
                nc.scalar.dma_start(mw0t[:, 256:H], mw0_d[b][:, 256:H])
            else:
                nc.gpsimd.dma_start(mw0t[:], mw0_d[b])
            mw1ft = mw1fp.tile([128, NF1, H], f8, tag="mw1f", bufs=4,
                               name=f"mw1f_{b}")
            mw1ht = mw1hp.tile([128, KT - NF1, H], f16, tag="mw1h", bufs=4,
                               name=f"mw1h_{b}")
            if b == 0:
                # mask 0's fp8 L1 weights follow mw0 on the scalar queue;
                # its fp16 L1 weights lead the gpsimd queue (interleaved
                # with the L2 stream below), keeping sync free for xyT and
                # mask 1's stream
                nc.scalar.dma_start(mw1ft[:, 0:2, :], mw1f_d[b][:, 0:2, :])
                nc.scalar.dma_start(mw1ft[:, 2:4, :], mw1f_d[b][:, 2:4, :])
                nc.gpsimd.dma_start(mw1ht[:, 0:2, :], mw1h_d[b][:, 0:2, :])
                nc.gpsimd.dma_start(mw1ht[:, 2:4, :], mw1h_d[b][:, 2:4, :])
            elif b == 1:
                nc.scalar.dma_start(mw1ft[:], mw1f_d[b])
                nc.scalar.dma_start(mw1ht[:, 0:2, :], mw1h_d[b][:, 0:2, :])
                nc.sync.dma_start(mw1ht[:, 2:4, :], mw1h_d[b][:, 2:4, :])
            else:
                nc.sync.dma_start(mw1ft[:], mw1f_d[b])
                nc.sync.dma_start(mw1ht[:], mw1h_d[b])
            # mw2/mw3 ride the gpsimd SWDGE queue (no compute duties)
            mw2ft = mw2p.tile([128, KT, H], f8, tag="mw2f", bufs=4,
                              name=f"mw2f_{b}")
            mw2ht = None
            if b < UPG:
                if b < 2:
                    nc.gpsimd.dma_start(mw2ft[:, 0:4, :],
                                        mw2fA_d[b][:, 0:4, :])
                    nc.gpsimd.dma_start(mw2ft[:, 4:8, :],
                                        mw2fA_d[b][:, 4:8, :])
                else:
                    nc.gpsimd.dma_start(mw2ft[:], mw2fA_d[b])
            else:
                mw2ht = mw2p.tile([128, 4, H], f16, tag="mw2h", bufs=4,
                                  name=f"mw2h_{b}")
                nc.gpsimd.dma_start(mw2ft[:, 0:4, :], mw2fB_d[b - UPG])
                nc.gpsimd.dma_start(mw2ht[:], mw2hB_d[b - UPG])
            mw3t = mw3p.tile([128, KT, D_OUT], f16, tag="mw3", bufs=4,
                             name=f"mw3_{b}")
            nc.gpsimd.dma_start(mw3t[:], mw3_d[b])
            return mw0t, mw1ft, mw1ht, mw2ft, mw2ht, mw3t

        # psum -> sbuf drain: all scales folded into weights/biases, so
        # every drain is max(psum + bias, 0), alternating Scalar/Vector
        def drain(at, ps, col, dve):
            if dve:
                nc.vector.tensor_scalar(at, ps, bt[:, col:col + 1], 0.0,
                                        add_op, max_op)
            else:
                nc.scalar.activation(at, ps, relu, bias=bt[:, col:col + 1])

        def drain_split(at, ps, col):
            # scalar runs ~3x slower per element than vector on fp16
            # drains, so give it the smaller share
            h = NLOC // 4
            nc.scalar.activation(at[:, 0:h], ps[:, 0:h], relu,
                                 bias=bt[:, col:col + 1])
            nc.vector.tensor_scalar(at[:, h:NLOC], ps[:, h:NLOC],
                                    bt[:, col:col + 1], 0.0,
                                    add_op, max_op)

        ps3 = ps3pool.tile([128, NLOC], f32, tag="ps3")

        def layer3_pairs(b, mw3t, a2t, kps):
            for kp in kps:
                k0, k1 = 2 * kp, 2 * kp + 1
                st = (b == 0 and kp == 0)
                sp = (b == B - 1 and kp == KT // 2 - 1)
                nc.tensor.matmul(ps3[0:D_OUT, :], mw3t[:, k0, :],
                                 a2t[:, k0, :],
                                 start=st, stop=sp, tile_position=(0, 0))
                nc.tensor.matmul(ps3[D_OUT:128, :], mw3t[:, k1, :],
                                 a2t[:, k1, :],
                                 start=st, stop=sp, tile_position=(0, 64))

        # ---- PE DVFS warmup: slow fp32 matmuls on the early bias block
        # keep the tensor engine busy through the startup DMA window
        wps = pspool.tile([128, NLOC], f32, tag="ps", name="warm")
        for i in range(NWARM):
            nc.tensor.matmul(wps[0:8, 0:40], bt[:, 0:8], bt[:, 0:40],
                             start=True, stop=True)

        fetched = {0: fetch(0)}
        fetched[1] = fetch(1)

        prev = None  # (b, mw3t, a2) pending layer-3
        for b in range(B):
            if b + 2 < B:
                fetched[b + 2] = fetch(b + 2)
            mw0t, mw1ft, mw1ht, mw2ft, mw2ht, mw3t = fetched.pop(b)

            # ---- layer 0: [96] -> [1024]; m<NF1 drains fp8 (psum carries
            # SA0 via mw0 cols), m>=NF1 drains fp16
            a0f = apool.tile([128, NF1, NLOC], f8, tag="a0f",
                             name=f"a0f_{b}")
            a0h = apool.tile([128, KT - NF1, NLOC], f16, tag="a0h",
                             name=f"a0h_{b}")
            for m in range(KT):
                ps = pspool.tile([128, NLOC], f32, tag="ps",
                                 name=f"ps_a0_{b}_{m}")
                nc.tensor.matmul(ps[:], mw0t[:, m * 128:(m + 1) * 128],
                                 xyT[:], start=True, stop=True)
                if m < NF1:
                    drain_split(a0f[:, m, :], ps[:], m)
                else:
                    drain_split(a0h[:, m - NF1, :], ps[:], m)

            if prev is not None:
                layer3_pairs(*prev, range(KT // 2))

            # ---- layer 1: 2 fp8 DR pairs + 4 fp16 k-slices per m-tile
            a1f = apool.tile([128, KT, NLOC], f8, tag="a1f",
                             name=f"a1f_{b}")
            a1h = apool.tile([128, KT - 4, NLOC], f16, tag="a1h",
                             name=f"a1h_{b}")
            nf2 = KT if b < UPG else 4
            groups = ([range(2 * g, 2 * g + 2) for g in range(4)]
                      if b == 0 else [range(0, 4), range(4, 8)])
            for ms in groups:
                pss = [pspool.tile([128, NLOC], f32, tag="ps",
                                   name=f"ps_a1_{b}_{m}") for m in ms]
                for t in range(NF1 // 2):
                    for mi, m in enumerate(ms):
                        nc.tensor.matmul(pss[mi][:],
                                         mw1ft[:, 2 * t:2 * t + 2,
                                               m * 128:(m + 1) * 128],
                                         a0f[:, 2 * t:2 * t + 2, :],
                                         start=(t == 0), stop=False,
                                         perf_mode=DR)
                for k in range(NF1, KT):
                    for mi, m in enumerate(ms):
                        nc.tensor.matmul(pss[mi][:],
                                         mw1ht[:, k - NF1,
                                               m * 128:(m + 1) * 128],
                                         a0h[:, k - NF1, :],
                                         start=False, stop=(k == KT - 1))
                for mi, m in enumerate(ms):
                    col = 8 + 16 * b + m
                    if m < nf2:
                        drain(a1f[:, m, :], pss[mi][:], col, dve=(m % 2 == 1))
                    else:
                        drain(a1h[:, m - 4, :], pss[mi][:], col,
                              dve=(m % 2 == 1))

            # ---- layer 2: group A 4 fp8 DR pairs; group B 2 DR + 4 fp16
            a2t = apool.tile([128, KT, NLOC], f16, tag="a2",
                             name=f"a2_{b}")
            for half in range(2):
                ms = range(half * 4, half * 4 + 4)
                pss = [pspool.tile([128, NLOC], f32, tag="ps",
                                   name=f"ps_a2_{b}_{m}") for m in ms]
                for t in range(nf2 // 2):
                    for mi, m in enumerate(ms):
                        nc.tensor.matmul(pss[mi][:],
                                         mw2ft[:, 2 * t:2 * t + 2,
                                               m * 128:(m + 1) * 128],
                                         a1f[:, 2 * t:2 * t + 2, :],
                                         start=(t == 0),
                                         stop=(t == nf2 // 2 - 1),
                                         perf_mode=DR)
                for k in range(nf2, KT):
                    for mi, m in enumerate(ms):
                        nc.tensor.matmul(pss[mi][:],
                                         mw2ht[:, k - 4,
                                               m * 128:(m + 1) * 128],
                                         a1h[:, k - 4, :],
                                         start=False, stop=(k == KT - 1))
                for mi, m in enumerate(ms):
                    col = 8 + 16 * b + 8 + m
                    if b == B - 1:
                        # last mask: drain-split each m-pair and issue its
                        # ps3 accumulation immediately -- shortest tail
                        drain_split(a2t[:, m, :], pss[mi][:], col)
                        if m % 2 == 1:
                            layer3_pairs(b, mw3t, a2t, [m // 2])
                    else:
                        drain(a2t[:, m, :], pss[mi][:], col,
                              dve=(m % 2 == 1))

            prev = (b, mw3t, a2t) if b < B - 1 else None

        # ---- finalize: fp16-copy ps3 in column halves, DMA out; host sums
        # the two 64-row col-group halves, scales and adds b3
        s3 = finp.tile([128, NLOC], f16, tag="s3")
        hh = NLOC // 2
        nc.scalar.copy(s3[:, 0:hh], ps3[:, 0:hh])
        nc.sync.dma_start(out_d[:, 0:hh], s3[:, 0:hh])
        nc.scalar.copy(s3[:, hh:NLOC], ps3[:, hh:NLOC])
        nc.sync.dma_start(out_d[:, hh:NLOC], s3[:, hh:NLOC])

    nc.compile()
    return nc


def _q8(v):
    return np.clip(v, -240.0, 240.0).astype(E4)


def _quant_weights(wt1, wt2, m1, m2, g1, g2):
    """Quantized L1/L2 weights with per-output-neuron gains folded in.
    Returns flat [B, 1024, H] fp32 dequantized views (for calibration)
    plus the raw quantized arrays."""
    w1q8, w1q16, w2q8, w2q16 = [], [], [], []
    k1 = NF1 * 128
    for b in range(B):
        s1 = (1.0 - g1[b])[None, :]
        w1q8.append(_q8(wt1[:k1] * m1[b, :k1] * SW1 * s1))
        w1q16.append((np.asarray(wt1[k1:], F16).astype(np.float32)
                      * m1[b, k1:] * SA1 * s1).astype(F16))
        s2 = (1.0 - g2[b])[None, :]
        k2 = H if b < UPG else 512
        w2q8.append(_q8(wt2[:k2] * m2[b, :k2] * SW2 * s2))
        w2q16.append((np.asarray(wt2[k2:], F16).astype(np.float32)
                      * m2[b, k2:] * SW2 * s2).astype(F16))
    return w1q8, w1q16, w2q8, w2q16


def _calibrate(W0, W1, W2, W3, b0, b1, b2, b3, m0, m1, m2, m3):
    """Gain+bias calibration over CALIB_M gaussian samples (input-
    independent). Two passes: pass 0 fits per-neuron gains g and biases,
    pass 1 refits biases with the gains folded into the weights.
    Returns (w1q8, w1q16, w2q8, w2q16, d1, d2, d3)."""
    rng = np.random.default_rng(7)
    xc = rng.standard_normal((D_IN, CALIB_M)).astype(np.float32)
    xc = xc.astype(F16).astype(np.float32)
    wt0_16 = np.asarray(W0.T, F16).astype(np.float32)
    wt1 = W1.T.astype(np.float32)
    wt2 = W2.T.astype(np.float32)
    wt3 = W3.T.astype(np.float32)
    wt3_16 = np.asarray(W3.T, F16).astype(np.float32)
    b0f = b0.astype(np.float32)[:, None]
    b1f = b1.astype(np.float32)[:, None]
    b2f = b2.astype(np.float32)[:, None]
    g1 = np.zeros((B, H), np.float32)
    g2 = np.zeros((B, H), np.float32)
    d1 = np.zeros((B, H), np.float32)
    d2 = np.zeros((B, H), np.float32)
    d3 = np.zeros(D_OUT, np.float32)
    k1 = NF1 * 128
    for it in range(2):
        d3[:] = 0
        qw = _quant_weights(wt1, wt2, m1, m2, g1, g2)
        for b in range(B):
            w1 = np.concatenate([qw[0][b].astype(np.float32) / SW1,
                                 qw[1][b].astype(np.float32) / SA1])
            w2 = np.concatenate([qw[2][b].astype(np.float32) / SW2,
                                 qw[3][b].astype(np.float32) / SW2])
            z0 = (wt0_16 * m0[b]).T @ xc + b0f
            a0r = np.maximum(z0, 0.0)
            a0q = np.concatenate([
                _q8(a0r[:k1] * SA0).astype(np.float32) / SA0,
                a0r[k1:].astype(F16).astype(np.float32)])
            z1r = (wt1 * m1[b]).T @ a0r + b1f
            z1q = w1.T @ a0q + b1f
            dz = z1r - z1q
            if it == 0:
                zc = z1q - z1q.mean(axis=1, keepdims=True)
                g1[b] += -(dz * zc).sum(axis=1) / np.maximum(
                    (zc * zc).sum(axis=1), 1e-9)
            d1[b] = dz.mean(axis=1)
            z1q = z1q + d1[b][:, None]
            a1r = np.maximum(z1r, 0.0)
            a1q = np.maximum(z1q, 0.0)
            k2 = H if b < UPG else 512
            a1q = np.concatenate([
                _q8(a1q[:k2] * SA1).astype(np.float32) / SA1,
                a1q[k2:].astype(F16).astype(np.float32)])
            z2r = (wt2 * m2[b]).T @ a1r + b2f
            z2q = w2.T @ a1q + b2f
            dz = z2r - z2q
            if it == 0:
                zc = z2q - z2q.mean(axis=1, keepdims=True)
                g2[b] += -(dz * zc).sum(axis=1) / np.maximum(
                    (zc * zc).sum(axis=1), 1e-9)
            d2[b] = dz.mean(axis=1)
            z2q = z2q + d2[b][:, None]
            a2r = np.maximum(z2r, 0.0)
            a2q = np.maximum(z2q, 0.0).astype(F16).astype(np.float32)
            o3r = (wt3 * m3[b]).T @ a2r
            o3q = (wt3_16 * m3[b]).T @ a2q
            d3 += (o3r - o3q).mean(axis=1) / B
    return qw, d1, d2, d3


def _prep_shared(W0, W1, W2, W3, b0, b1, b2, b3,
                 mask0, mask1, mask2, mask3):
    def mfold(m, out_w):
        # [B_, 1024, out] -> [B_, 128, KT_, out]
        b_ = m.shape[0]
        kt = m.shape[1] // 128
        return np.ascontiguousarray(
            m.reshape(b_, kt, 128, out_w).transpose(0, 2, 1, 3))

    qw, d1, d2, d3 = _calibrate(W0, W1, W2, W3, b0, b1, b2, b3,
                                mask0, mask1, mask2, mask3)

    wt0 = np.asarray(W0.T, F16).astype(np.float32)
    mw0 = (wt0[None] * mask0).copy()
    mw0[:, :, :NF1 * 128] *= SA0
    # pad the contraction dim to 128 rows (zeros): full-height stationary
    # tiles stream ~25ns/instr faster than 96-row ones
    mw0 = np.concatenate(
        [mw0, np.zeros((B, 128 - D_IN, H), np.float32)], axis=1)
    mw0 = mw0.astype(F16)                                      # [B, 128, H]
    mw1f = mfold(np.stack(qw[0]), H)                           # [B,128,4,H]
    mw1h = mfold(np.stack(qw[1]), H)                           # [B,128,4,H]
    mw2fA = mfold(np.stack(qw[2][:UPG]), H)                    # [UPG,128,8,H]
    mw2fB = mfold(np.stack(qw[2][UPG:]), H)                    # [.,128,4,H]
    mw2hB = mfold(np.stack(qw[3][UPG:]), H)                    # [.,128,4,H]
    wt3 = np.asarray(W3.T, F16).astype(np.float32)
    mw3 = mfold((wt3[None] * mask3).astype(F16), D_OUT)        # [B,128,8,64]

    def brs(v):
        return np.ascontiguousarray(
            v.reshape(KT, 128).T).astype(np.float32)

    b0s = brs(b0.astype(np.float32))
    b0s[:, :NF1] *= SA0
    cols = [b0s]
    for b in range(B):
        cols.append(brs(SA1 * (b1.astype(np.float32) + d1[b])))
        cols.append(brs(SA1 * SW2 * (b2.astype(np.float32) + d2[b])))
    biasp = np.concatenate(cols, axis=1)                       # [128, 264]
    b3r = b3.astype(np.float32) + d3

    return dict(
        mw0=mw0, mw1f=mw1f, mw1h=mw1h, mw2fA=mw2fA, mw2fB=mw2fB,
        mw2hB=mw2hB, mw3=mw3, biasp=biasp), b3r


def kernel(xy, W0, b0, W1, b1, W2, b2, W3, b3,
           mask0, mask1, mask2, mask3):
    global LAST_RESULT
    xy = np.asarray(xy, np.float32)
    args = [np.asarray(a, np.float32) for a in
            (W0, W1, W2, W3, b0, b1, b2, b3)]
    masks = [np.asarray(m, np.float32) for m in (mask0, mask1, mask2, mask3)]

    if "nc" not in _CACHE:
        _CACHE["nc"] = _build_graph()
    nc = _CACHE["nc"]

    pkey = tuple(id(a) for a in args + masks)
    if _CACHE.get("pkey") != pkey:
        _CACHE["prep"] = _prep_shared(*args, *masks)
        _CACHE["pkey"] = pkey
    shared, b3r = _CACHE["prep"]

    xyT = np.concatenate(
        [xy.T, np.zeros((128 - D_IN, N), np.float32)], axis=0)
    xyT = np.ascontiguousarray(xyT).astype(F16)    # [128, 4096]
    in_maps = []
    for core in range(NCORES):
        im = dict(shared)
        im["xyT"] = np.ascontiguousarray(
            xyT[:, core * NLOC:(core + 1) * NLOC])
        in_maps.append(im)

    if TRACE:
        _ensure_ntff_hook()
    res = run_bass_kernel_spmd(
        nc, in_maps, core_ids=list(range(NCORES)),
        trace=TRACE)
    LAST_RESULT = res
    outs = []
    for i in range(NCORES):
        s3 = np.asarray(res.results[i]["out"], np.float32)     # [128, NLOC]
        o = (s3[:D_OUT] + s3[D_OUT:]) / (SA1 * SW2 * B) + b3r[:, None]
        outs.append(o.T)
    return np.concatenate(outs, axis=0).astype(np.float32)
